# revision 16
# baseline (speedup 1.0000x reference)
"""DCRNN decoder (2-layer DCGRU + projection) on 8 trn2 NeuronCores.

Sharding: data-parallel over batch (B=32 -> 4 per core); supports + weights
replicated. Per-core kernel computes the full 2-cell stack for its 4 batches.

Math (per feature-column block, matching the reference's x0/x1 rebinding):
  z1 = S0@x0 ; z2raw = S0@z1 ; z3 = S1@z1 ; z4raw = S1@z3
  gconv = x0@(W0-W2) + z1@(W1-W4) + z2raw@(2W2) + z3@W3 + z4raw@(2W4)  [+bias]
with S0 = rw(A).T, S1 = rw(A.T).T.  On-chip we hold S^T matrices (rw(A),
rw(A.T)) and run diffusion in "transposed" orientation:
  ZT[fb, n] = sum_k x0[k, fb] * ST[k, n]   (lhsT = x0 n-major, rhs = ST panels)
which lands results feature-major, exactly what the dense stage contracts.

Layouts per core (b=4 local batches, N=2048, U=64, P=128):
 - n-major lhsT tiles [128 n-part, ..., f] bf16; 64-wide feature blocks are
   pair-packed two batches per tile (b even -> free cols 0:64, odd -> 64:128).
 - T-form [f-part, n-free], pair-packed on partitions ((b0|b1) 64-wide).
 - dense: psum[128, 512]; weight pieces are zero-padded on both the row
   (pair-half select) and column (partition placement) axes so each output
   lands on its own partition half, keeping every eviction lane-aligned.
 - L1 chains run in two batch-halves to bound SBUF.
"""
import os
import sys
import time

sys.path.insert(0, "/opt/trn_rl_repo")

import numpy as np
import ml_dtypes

import concourse.bass as bass
import concourse.bacc as bacc
import concourse.tile as tile
import concourse.mybir as mybir
from concourse.masks import make_identity

P = 128
N = 2048
NT = 16
U = 64
NB = 4
MM = mybir.dt.bfloat16
F32 = mybir.dt.float32
NPMM = ml_dtypes.bfloat16
ACT = mybir.ActivationFunctionType


def _build():
    nc = bacc.Bacc("TRN2", target_bir_lowering=False, debug=False)

    d = {}
    def din(name, shape, dt):
        d[name] = nc.dram_tensor(name, shape, dt, kind="ExternalInput").ap()
    def dout(name, shape, dt):
        d[name] = nc.dram_tensor(name, shape, dt, kind="ExternalOutput").ap()

    din("s0t", (N, N), MM)
    din("s1t", (N, N), MM)
    din("xinp", (N, NB), MM)
    din("h0prev", (NB, N, U), F32)
    din("h1prev", (NB, N, U), F32)
    for wn in ("wg0r", "wg0u", "wc0x", "wg1r", "wg1u", "wc1x"):
        din(wn, (5, 2, P, P), MM)
    for wn in ("w5gr", "w5gu", "w5cx"):
        din(wn, (2, P, P), MM)
    for bn in ("bg0r", "bg0u", "bc0", "bg1r", "bg1u", "bc1"):
        din(bn, (P, 1), F32)
    din("wproj2", (2, P, 1), F32)
    din("bproj", (1, 1), F32)
    dout("h0o", (NB, N, U), F32)
    dout("h1o", (NB, N, U), F32)
    dout("outp", (NB, N), F32)

    from contextlib import ExitStack
    with tile.TileContext(nc) as tc, ExitStack() as ctx:
        wp = ctx.enter_context(tc.tile_pool(name="wts", bufs=1))
        sp = ctx.enter_context(tc.tile_pool(name="small", bufs=1))
        wk16 = ctx.enter_context(tc.tile_pool(name="wk16", bufs=5))
        wk8 = ctx.enter_context(tc.tile_pool(name="wk8", bufs=8))
        itp = ctx.enter_context(tc.tile_pool(name="itp", bufs=2))
        pan = ctx.enter_context(tc.tile_pool(name="pan", bufs=5))
        psp = ctx.enter_context(tc.tile_pool(name="psum", bufs=8, space="PSUM"))

        def w16(shape, dt=F32):
            return wk16.tile(shape, dt, tag="w16", name="w16")

        def w8(shape, dt=MM):
            return wk8.tile(shape, dt, tag="w8", name="w8")

        def pst(dt=F32):
            return psp.tile([P, 512], dt, tag="ps", name="ps")

        # ---- weights/biases to SBUF ----
        wsb = {}
        for wn in ("wg0r", "wg0u", "wc0x", "wg1r", "wg1u", "wc1x"):
            wsb[wn] = wp.tile([P, 5, 2, P], MM, tag=wn, name=wn)
            nc.sync.dma_start(wsb[wn][:], d[wn].rearrange("m v p o -> p m v o"))
        for wn in ("w5gr", "w5gu", "w5cx"):
            wsb[wn] = wp.tile([P, 2, P], MM, tag=wn, name=wn)
            nc.sync.dma_start(wsb[wn][:], d[wn].rearrange("v p o -> p v o"))
        bias = {}
        for bn in ("bg0r", "bg0u", "bc0", "bg1r", "bg1u", "bc1"):
            bias[bn] = wp.tile([P, 1], F32, tag=bn, name=bn)
            nc.sync.dma_start(bias[bn][:], d[bn])
        wproj2 = wp.tile([P, 2, 1], F32, tag="wproj2")
        nc.sync.dma_start(wproj2[:], d["wproj2"].rearrange("v p o -> p v o"))
        bproj = wp.tile([1, 1], F32, tag="bproj")
        nc.sync.dma_start(bproj[:], d["bproj"])

        idmm = sp.tile([P, P], MM, tag="idmm")
        make_identity(nc, idmm[:])
        idf = sp.tile([P, P], F32, tag="idf")
        make_identity(nc, idf[:])

        # ---------- helpers ----------
        def diff_pass(dst, nfb, lhs_fn, mat, inp_lhs_fn=None, inp_dst=None):
            for ngh in range(2):
                npsum = nfb + (1 if inp_lhs_fn else 0)
                pss = [[pst() for _ in range(2)] for _ in range(npsum)]
                for k in range(NT):
                    pt = pan.tile([P, 1024], MM, tag="pan", name="pan")
                    nc.sync.dma_start(
                        pt[:], mat[k * P:(k + 1) * P, ngh * 1024:(ngh + 1) * 1024])
                    for ti in range(nfb):
                        lhsT = lhs_fn(ti, k)
                        for j in range(2):
                            nc.tensor.matmul(
                                pss[ti][j][:, :], lhsT, pt[:, j * 512:(j + 1) * 512],
                                start=(k == 0), stop=(k == NT - 1))
                    if inp_lhs_fn is not None:
                        lhsT = inp_lhs_fn(k)
                        for j in range(2):
                            nc.tensor.matmul(
                                pss[nfb][j][:4, :], lhsT, pt[:, j * 512:(j + 1) * 512],
                                start=(k == 0), stop=(k == NT - 1))
                for ti in range(nfb):
                    for j in range(2):
                        c0 = ngh * 1024 + j * 512
                        nc.vector.tensor_copy(dst[:, ti, c0:c0 + 512], pss[ti][j][:, :])
                if inp_dst is not None:
                    for j in range(2):
                        c0 = ngh * 1024 + j * 512
                        nc.vector.tensor_copy(inp_dst[0:4, c0:c0 + 512],
                                              pss[nfb][j][:4, :])

        def zt_to_n(zn, zt, nfb):
            for t in range(nfb):
                for nt in range(NT):
                    ps = pst(MM)
                    nc.tensor.transpose(ps[:, :P], zt[:, t, nt * P:(nt + 1) * P],
                                        idmm[:, :])
                    nc.vector.tensor_copy(zn[:, t, nt, :], ps[:, :P])

        def inp_to_n(dstn, srcT):
            for nt in range(NT):
                ps = pst(MM)
                nc.tensor.transpose(ps[:, :4], srcT[0:4, nt * P:(nt + 1) * P],
                                    idmm[0:4, 0:4])
                nc.vector.tensor_copy(dstn[:, nt, :], ps[:, :4])

        # ================= LAYER 0 =================
        hx0 = w16([P, 2, NT, 2, U])
        for pr in range(2):
            for nt in range(NT):
                nc.sync.dma_start(
                    hx0[:, pr, nt],
                    d["h0prev"][2 * pr:2 * pr + 2, nt * P:(nt + 1) * P, :]
                    .rearrange("b p u -> p b u"))
        x0h = w8([P, 2, NT, P])
        for pr in range(2):
            nc.vector.tensor_copy(
                x0h[:, pr].rearrange("p t (b u) -> p t b u", b=2), hx0[:, pr])
        hT = w16([P, 2, N])      # fp32 T-form pairs (b0u | b1u)
        x0hT = w8([P, 2, N])
        for pr in range(2):
            for nt in range(NT):
                ps = pst()
                nc.tensor.transpose(
                    ps[:, :P], hx0[:, pr, nt].rearrange("p b u -> p (b u)"), idf[:, :])
                nc.vector.tensor_copy(hT[:, pr, nt * P:(nt + 1) * P], ps[:, :P])
                nc.vector.tensor_copy(x0hT[:, pr, nt * P:(nt + 1) * P], ps[:, :P])

        x0inp = sp.tile([P, NT, 4], MM, tag="x0inp")
        for nt in range(NT):
            nc.sync.dma_start(x0inp[:, nt, :], d["xinp"][nt * P:(nt + 1) * P, :])
        XinpT = w16([P, NB, N], MM)
        nc.any.memzero(XinpT[:])
        x0inpT = sp.tile([4, N], MM, tag="x0inpT")
        for nt in range(NT):
            ps = pst(MM)
            nc.tensor.transpose(ps[:4, :P], x0inp[:, nt, :], idmm[:, :])
            nc.vector.tensor_copy(x0inpT[0:4, nt * P:(nt + 1) * P], ps[:4, :P])
        nc.sync.dma_start(XinpT[0:1, :, :], x0inpT[0:4, :])

        def l0_chain(x0_lhs_fn, inp, dense_fn):
            zts = [w8([P, 2, N]) for _ in range(4)]
            itmp = [itp.tile([4, N], MM, tag="itmp", name="itmp") for _ in range(4)] \
                if inp else [None] * 4
            zin = [None, None]
            diff_pass(zts[0], 2, x0_lhs_fn, d["s0t"],
                      (lambda k: x0inp[:, k, :]) if inp else None, itmp[0])
            z1n = w8([P, 2, NT, P])
            zt_to_n(z1n, zts[0], 2)
            if inp:
                nc.sync.dma_start(XinpT[1:2, :, :], itmp[0][0:4, :])
                zin[0] = sp.tile([P, NT, 4], MM, tag="zin0", name="zin0")
                inp_to_n(zin[0], itmp[0])
            diff_pass(zts[1], 2, lambda t, k: z1n[:, t, k, :], d["s0t"],
                      (lambda k: zin[0][:, k, :]) if inp else None, itmp[1])
            if inp:
                nc.sync.dma_start(XinpT[2:3, :, :], itmp[1][0:4, :])
            diff_pass(zts[2], 2, lambda t, k: z1n[:, t, k, :], d["s1t"],
                      (lambda k: zin[0][:, k, :]) if inp else None, itmp[2])
            z3n = w8([P, 2, NT, P])
            zt_to_n(z3n, zts[2], 2)
            if inp:
                nc.sync.dma_start(XinpT[3:4, :, :], itmp[2][0:4, :])
                zin[1] = sp.tile([P, NT, 4], MM, tag="zin1", name="zin1")
                inp_to_n(zin[1], itmp[2])
            diff_pass(zts[3], 2, lambda t, k: z3n[:, t, k, :], d["s1t"],
                      (lambda k: zin[1][:, k, :]) if inp else None, itmp[3])
            if inp:
                nc.sync.dma_start(XinpT[4:5, :, :], itmp[3][0:4, :])
            dense_fn(zts)

        def dense_l0(zts, m0T, jobs):
            """jobs: list of (w_name, w5_name, bias_name, actfn, dstT[P,2,N])"""
            for b in range(NB):
                pr, v = b // 2, b % 2
                lo = slice(v * U, (v + 1) * U)
                for ng in range(4):
                    cs = slice(ng * 512, (ng + 1) * 512)
                    for (wn, w5n, bn, fn, dstT) in jobs:
                        ps = pst()
                        rhss = [m0T[:, pr, cs]] + [zts[m][:, pr, cs] for m in range(4)]
                        for m in range(5):
                            nc.tensor.matmul(ps[:, :], wsb[wn][:, m, v, :], rhss[m],
                                             start=(m == 0), stop=False)
                        nc.tensor.matmul(ps[:, :], wsb[w5n][:, v, :],
                                         XinpT[:, b, cs], start=False, stop=True)
                        nc.scalar.activation(dstT[lo, pr, cs], ps[lo, :], fn,
                                             bias=bias[bn][lo, 0:1])

        rT = w8([P, 2, N])
        uT = w16([P, 2, N])
        l0_chain(lambda t, k: x0h[:, t, k, :], True,
                 lambda zts: dense_l0(zts, x0hT,
                                      [("wg0r", "w5gr", "bg0r", ACT.Sigmoid, rT),
                                       ("wg0u", "w5gu", "bg0u", ACT.Sigmoid, uT)]))
        hTb = w8([P, 2, N])
        nc.vector.tensor_copy(hTb[:], hT[:])
        rhb = w8([P, 2, N])
        nc.vector.tensor_mul(rhb[:], rT[:], hTb[:])
        xhc = w8([P, 2, NT, P])
        for pr in range(2):
            for nt in range(NT):
                ps = pst(MM)
                nc.tensor.transpose(ps[:, :P], rhb[:, pr, nt * P:(nt + 1) * P],
                                    idmm[:, :])
                nc.vector.tensor_copy(xhc[:, pr, nt, :], ps[:, :P])
        cT = w16([P, 2, N])
        l0_chain(lambda t, k: xhc[:, t, k, :], False,
                 lambda zts: dense_l0(zts, rhb,
                                      [("wc0x", "w5cx", "bc0", ACT.Tanh, cT)]))
        # h0' = c + u*(h - c), computed in place into hT
        nc.vector.tensor_sub(hT[:], hT[:], cT[:])
        nc.vector.tensor_mul(hT[:], uT[:], hT[:])
        nc.vector.tensor_add(hT[:], hT[:], cT[:])

        # transpose h0' to n-major: fp32 to HBM + bf16 into x0l1 left half
        x0l1 = w16([P, NB, NT, P], MM)
        hn = w16([P, 2, NT, 2, U])
        for pr in range(2):
            for nt in range(NT):
                ps = pst()
                nc.tensor.transpose(ps[:, :P], hT[:, pr, nt * P:(nt + 1) * P],
                                    idf[:, :])
                nc.vector.tensor_copy(
                    hn[:, pr, nt].rearrange("p b u -> p (b u)"), ps[:, :P])
                for v in range(2):
                    nc.vector.tensor_copy(x0l1[:, 2 * pr + v, nt, 0:U],
                                          ps[:, v * U:(v + 1) * U])
            for nt in range(NT):
                nc.sync.dma_start(
                    d["h0o"][2 * pr:2 * pr + 2, nt * P:(nt + 1) * P, :]
                    .rearrange("b p u -> p b u"), hn[:, pr, nt])

        # ================= LAYER 1 =================
        hx1 = w16([P, 2, NT, 2, U])
        for pr in range(2):
            for nt in range(NT):
                nc.sync.dma_start(
                    hx1[:, pr, nt],
                    d["h1prev"][2 * pr:2 * pr + 2, nt * P:(nt + 1) * P, :]
                    .rearrange("b p u -> p b u"))
        hT1 = w16([P, 2, N])
        for pr in range(2):
            for nt in range(NT):
                for v in range(2):
                    nc.vector.tensor_copy(x0l1[:, 2 * pr + v, nt, U:2 * U],
                                          hx1[:, pr, nt, v])
                ps = pst()
                nc.tensor.transpose(
                    ps[:, :P], hx1[:, pr, nt].rearrange("p b u -> p (b u)"), idf[:, :])
                nc.vector.tensor_copy(hT1[:, pr, nt * P:(nt + 1) * P], ps[:, :P])

        def l1_chain(h, x0_lhs_fn, dense_fn):
            zts = [w8([P, 2, N]) for _ in range(4)]
            diff_pass(zts[0], 2, x0_lhs_fn, d["s0t"])
            z1n = w8([P, 2, NT, P])
            zt_to_n(z1n, zts[0], 2)
            diff_pass(zts[1], 2, lambda t, k: z1n[:, t, k, :], d["s0t"])
            diff_pass(zts[2], 2, lambda t, k: z1n[:, t, k, :], d["s1t"])
            z3n = w8([P, 2, NT, P])
            zt_to_n(z3n, zts[2], 2)
            diff_pass(zts[3], 2, lambda t, k: z3n[:, t, k, :], d["s1t"])
            dense_fn(h, zts)

        def mk_x0T(h, src):
            x0T = w8([P, 2, N])
            for t in range(2):
                for nt in range(NT):
                    ps = pst(MM)
                    nc.tensor.transpose(ps[:, :P], src[:, 2 * h + t, nt], idmm[:, :])
                    nc.vector.tensor_copy(x0T[:, t, nt * P:(nt + 1) * P], ps[:, :P])
            return x0T

        def dense_l1(h, zts, m0T, jobs):
            for t in range(2):
                b = 2 * h + t
                v = t
                lo = slice(v * U, (v + 1) * U)
                for ng in range(4):
                    cs = slice(ng * 512, (ng + 1) * 512)
                    for (wn, bn, fn, dstT) in jobs:
                        ps = pst()
                        rhss = [m0T[:, t, cs]] + [zts[m][:, t, cs] for m in range(4)]
                        for m in range(5):
                            nc.tensor.matmul(ps[:, :], wsb[wn][:, m, v, :], rhss[m],
                                             start=(m == 0), stop=(m == 4))
                        nc.scalar.activation(dstT[lo, h, cs], ps[lo, :], fn,
                                             bias=bias[bn][lo, 0:1])

        rT1 = w8([P, 2, N])
        uT1 = w16([P, 2, N])
        for h in range(2):
            x0T = mk_x0T(h, x0l1)
            l1_chain(h, lambda t, k: x0l1[:, 2 * h + t, k, :],
                     lambda hh, zts: dense_l1(hh, zts, x0T,
                                              [("wg1r", "bg1r", ACT.Sigmoid, rT1),
                                               ("wg1u", "bg1u", ACT.Sigmoid, uT1)]))
        hTb1 = w8([P, 2, N])
        nc.vector.tensor_copy(hTb1[:], hT1[:])
        rhb1 = w8([P, 2, N])
        nc.vector.tensor_mul(rhb1[:], rT1[:], hTb1[:])
        # overwrite x0l1 cols 64:128 with r*h (n-major): cand input [h0 | rh]
        for pr in range(2):
            for nt in range(NT):
                ps = pst(MM)
                nc.tensor.transpose(ps[:, :P], rhb1[:, pr, nt * P:(nt + 1) * P],
                                    idmm[:, :])
                for v in range(2):
                    nc.vector.tensor_copy(x0l1[:, 2 * pr + v, nt, U:2 * U],
                                          ps[:, v * U:(v + 1) * U])
        cT1 = w16([P, 2, N])
        for h in range(2):
            x0T = mk_x0T(h, x0l1)
            l1_chain(h, lambda t, k: x0l1[:, 2 * h + t, k, :],
                     lambda hh, zts: dense_l1(hh, zts, x0T,
                                              [("wc1x", "bc1", ACT.Tanh, cT1)]))
        # h1' in place into hT1
        nc.vector.tensor_sub(hT1[:], hT1[:], cT1[:])
        nc.vector.tensor_mul(hT1[:], uT1[:], hT1[:])
        nc.vector.tensor_add(hT1[:], hT1[:], cT1[:])

        hn1 = w16([P, 2, NT, 2, U])
        for pr in range(2):
            for nt in range(NT):
                ps = pst()
                nc.tensor.transpose(ps[:, :P], hT1[:, pr, nt * P:(nt + 1) * P],
                                    idf[:, :])
                nc.vector.tensor_copy(
                    hn1[:, pr, nt].rearrange("p b u -> p (b u)"), ps[:, :P])
            for nt in range(NT):
                nc.sync.dma_start(
                    d["h1o"][2 * pr:2 * pr + 2, nt * P:(nt + 1) * P, :]
                    .rearrange("b p u -> p b u"), hn1[:, pr, nt])

        # ---- projection ----
        outrow = sp.tile([1, N], F32, tag="outrow")
        for pr in range(2):
            for v in range(2):
                b = 2 * pr + v
                for ng in range(4):
                    cs = slice(ng * 512, (ng + 1) * 512)
                    ps = pst()
                    nc.tensor.matmul(ps[:1, :], wproj2[:, v, :], hT1[:, pr, cs],
                                     start=True, stop=True)
                    nc.vector.tensor_scalar_add(outrow[0:1, cs], ps[:1, :],
                                                bproj[0:1, 0:1])
                nc.sync.dma_start(d["outp"][b:b + 1, :], outrow[0:1, :])

    nc.compile()
    return nc


def _fold(w, F, O):
    Wr = np.asarray(w, np.float32).reshape(F, 5, O)
    p = [Wr[:, m].copy() for m in range(5)]
    return [p[0] - p[2], p[1] - p[4], 2.0 * p[2], p[3], 2.0 * p[4]]


def _prep_maps(inputs):
    A = np.asarray(inputs["adj_mx"], np.float32)
    S0T = A / np.maximum(A.sum(1, keepdims=True), 1e-8)
    At = np.ascontiguousarray(A.T)
    S1T = At / np.maximum(At.sum(1, keepdims=True), 1e-8)

    xin = np.asarray(inputs["inputs"], np.float32)
    h0p = np.asarray(inputs["hidden_state"][0], np.float32).reshape(32, N, U)
    h1p = np.asarray(inputs["hidden_state"][1], np.float32).reshape(32, N, U)

    wg0p = _fold(inputs["w_gate_0"], 65, 2 * U)
    wc0p = _fold(inputs["w_cand_0"], 65, U)
    wg1p = _fold(inputs["w_gate_1"], 2 * U, 2 * U)
    wc1p = _fold(inputs["w_cand_1"], 2 * U, U)

    def pairw(pieces, rows, cols, full_rows):
        """(5,2,P,P): [m,v, rowplace, 64v:(v+1)*64 cols] = pieces[m][rows, cols]"""
        w = np.zeros((5, 2, P, P), np.float32)
        for m in range(5):
            blk = pieces[m][rows, cols]
            for v in range(2):
                if full_rows:
                    w[m, v, :, v * U:(v + 1) * U] = blk
                else:
                    w[m, v, v * U:(v + 1) * U, v * U:(v + 1) * U] = blk
        return w.astype(NPMM)

    rs = slice(1, 65)
    wg0r = pairw(wg0p, rs, slice(0, U), False)
    wg0u = pairw(wg0p, rs, slice(U, 2 * U), False)
    wc0x = pairw(wc0p, rs, slice(0, U), False)
    wg1r = pairw(wg1p, slice(0, P), slice(0, U), True)
    wg1u = pairw(wg1p, slice(0, P), slice(U, 2 * U), True)
    wc1x = pairw(wc1p, slice(0, P), slice(0, U), True)

    def w5x(pieces, cols):
        w = np.zeros((2, P, P), np.float32)
        for v in range(2):
            for m in range(5):
                w[v, m, v * U:(v + 1) * U] = pieces[m][0, cols]
        return w.astype(NPMM)

    w5gr = w5x(wg0p, slice(0, U))
    w5gu = w5x(wg0p, slice(U, 2 * U))
    w5cx = w5x(wc0p, slice(0, U))

    def bpair(x):
        v = np.asarray(x, np.float32).reshape(-1)
        return np.concatenate([v, v]).reshape(P, 1)

    bg0 = np.asarray(inputs["b_gate_0"], np.float32)
    bg1 = np.asarray(inputs["b_gate_1"], np.float32)

    wproj = np.asarray(inputs["w_proj"], np.float32).reshape(U)
    wproj2 = np.zeros((2, P, 1), np.float32)
    for v in range(2):
        wproj2[v, v * U:(v + 1) * U, 0] = wproj

    shared = {
        "s0t": S0T.astype(NPMM), "s1t": S1T.astype(NPMM),
        "wg0r": wg0r, "wg0u": wg0u, "wc0x": wc0x,
        "wg1r": wg1r, "wg1u": wg1u, "wc1x": wc1x,
        "w5gr": w5gr, "w5gu": w5gu, "w5cx": w5cx,
        "bg0r": bpair(bg0[:U]), "bg0u": bpair(bg0[U:]),
        "bc0": bpair(inputs["b_cand_0"]),
        "bg1r": bpair(bg1[:U]), "bg1u": bpair(bg1[U:]),
        "bc1": bpair(inputs["b_cand_1"]),
        "wproj2": wproj2,
        "bproj": np.asarray(inputs["b_proj"], np.float32).reshape(1, 1),
    }
    maps = []
    for c in range(8):
        bs = slice(c * NB, (c + 1) * NB)
        m = dict(shared)
        m["xinp"] = np.ascontiguousarray(xin[bs].T).astype(NPMM)
        m["h0prev"] = np.ascontiguousarray(h0p[bs])
        m["h1prev"] = np.ascontiguousarray(h1p[bs])
        maps.append(m)
    return maps


_NC_CACHE = {}


def _get_nc():
    if "nc" not in _NC_CACHE:
        _NC_CACHE["nc"] = _build()
    return _NC_CACHE["nc"]


def kernel(**inputs):
    from concourse.bass_utils import run_bass_kernel_spmd
    nc = _get_nc()
    maps = _prep_maps(inputs)
    res = run_bass_kernel_spmd(nc, maps, list(range(8))).results
    B = 32
    out = np.zeros((B, N), np.float32)
    h0 = np.zeros((B, N * U), np.float32)
    h1 = np.zeros((B, N * U), np.float32)
    for c in range(8):
        bs = slice(c * NB, (c + 1) * NB)
        out[bs] = res[c]["outp"]
        h0[bs] = res[c]["h0o"].reshape(NB, N * U)
        h1[bs] = res[c]["h1o"].reshape(NB, N * U)
    return out, np.stack([h0, h1])


if __name__ == "__main__":
    if os.environ.get("BASS_SIM"):
        os.environ["JAX_PLATFORMS"] = "cpu"
        from concourse.bass_interp import CoreSim
        t0 = time.time()
        nc = _build()
        print(f"[{time.time()-t0:.1f}s] built")
        import proto
        import reference as R
        inputs = {k: np.asarray(v) for k, v in R.setup_inputs().items()}
        maps = _prep_maps(inputs)
        sim = CoreSim(nc, trace=False)
        for k, v in maps[0].items():
            sim.tensor(k)[:] = v
        t0 = time.time()
        sim.simulate(check_with_hw=False)
        print(f"[{time.time()-t0:.1f}s] simulated")
        eo, eh = proto.run_kernel_emu(inputs, proto.bf16)
        got = {"outp": np.asarray(sim.tensor("outp"), np.float32),
               "h0o": np.asarray(sim.tensor("h0o"), np.float32).reshape(NB, N * U),
               "h1o": np.asarray(sim.tensor("h1o"), np.float32).reshape(NB, N * U)}
        exp = {"outp": eo[:NB], "h0o": eh[0, :NB], "h1o": eh[1, :NB]}
        for k in got:
            aerr = np.abs(got[k] - exp[k]).max()
            print(f"{k}: absmax vs emu = {aerr:.3e}  (scale {np.abs(exp[k]).max():.3e})")
